# revision 69
# baseline (speedup 1.0000x reference)
"""Trainium2 Bass kernel for nn_ATTLayer (attention pooling).

Reference computation (full input [64, 512, 1024] fp32):
    wb    = attention_w + attention_b          # [1024, 256] (b broadcast over rows)
    u_t   = tanh(inputs @ wb)                  # [64, 512, 256]
    logit = u_t @ attention_u                  # [64, 512]
    w     = softmax(logit, axis=1)             # softmax over seq
    out   = sum_s w[:, s] * inputs[:, s, :]    # [64, 1024]

Sharding: data-parallel over batch — 8 batches per core on 8 NeuronCores, no
collectives. Tiny params (wb = W + b, u) are precomputed/replicated on host.

All heavy math runs on PE (HW-measured: back-to-back matmuls stream at full
rate; DVE reduces cost ~0.7us per [128,512] on HW, so the weighted sum stays
on PE). x is uploaded in TWO layouts (block-transposed xt for GEMM1, natural
x for the weighted sum) on separate HWDGE rings (~290 / ~220 GB/s each,
HW-measured, transfers run on DMA engines, not the issuing queue).

Per local batch b (two-level software pipelining keeps PE stall-free:
logit^T(b) is emitted after GEMM1(b+1), softmax-sum+step7(b) after
GEMM1(b+2), so the ACT-exp chain for b overlaps a full GEMM):
  1. DMA xt (SP ring), x natural (scalar ring)
  2. GEMM1: p_u[a] += wb[k,a].T @ xt[k], 8 k-chunks; tanh per a-half (ACT)
  3. logit^T [s_loc, 2t] via 8 small matmuls (ut chunk stationary, u padded
     with zero columns for the even-free-dim rule)
  4. wt = exp(logit^T) on ACT, unnormalized. No max-subtraction:
     |logit| <= ~20 so exp stays finite in fp32/bf16.
  5. softmax sum = ones.T @ wt (PE) -> DVE reduce -> reciprocal
  6. step7: p_o[1, h] += wt[:, t].T @ x[t-chunk, h]  (8 matmuls)
  7. evacuation applies the 1/sum scale (ACT half / DVE half); ONE output
     DMA for all batches at the end on the then-idle SP ring.

Startup notes (HW-measured): ~8.5us of engine program loads precede any
work; the warm tile is shipped as a tiny SP-ring input so PE warm-up only
waits on SP (a memset would chain behind the last engine's program load,
and any Q7 ext-ISA op would add a ~6us IRAM library load to startup).

bf16 matmul operands / fp32 accumulation end-to-end rel err ~7e-3.
"""

import numpy as np

N_CORES = 8
B_FULL = 64
B_LOC = B_FULL // N_CORES  # 8 batches per core
S = 512
H = 1024
A = 256
P = 128
NT = S // P      # 4 s-tiles per batch
NK = H // P      # 8 h-chunks
NA = A // P      # 2 a-chunks

_CACHE = {}


def _build():
    import concourse.bacc as bacc
    import concourse.mybir as mybir
    import concourse.tile as tile

    F32 = mybir.dt.float32
    BF16 = mybir.dt.bfloat16
    AF = mybir.ActivationFunctionType
    AX = mybir.AxisListType
    ALU = mybir.AluOpType

    nc = bacc.Bacc("TRN2", target_bir_lowering=False, debug=False)

    # xt is partition-major on the host: each partition's 8 k-chunks are one
    # 8KB contiguous run, so DMA descriptors cover 2-8KB lines (~1.5x rate)
    xt_d = nc.dram_tensor("xt", [B_LOC, P, NK, S], BF16, kind="ExternalInput").ap()
    x_d = nc.dram_tensor("x", [B_LOC, P, NT, H], BF16, kind="ExternalInput").ap()
    wb_d = nc.dram_tensor("wb", [H, A], BF16, kind="ExternalInput").ap()
    u_d = nc.dram_tensor("u4", [P, 2 * NA], BF16, kind="ExternalInput").ap()
    warm_d = nc.dram_tensor("warm", [P, P], BF16, kind="ExternalInput").ap()
    out_d = nc.dram_tensor("out", [B_LOC, H], F32, kind="ExternalOutput").ap()

    with tile.TileContext(nc) as tc:
        with (
            tc.tile_pool(name="const", bufs=1) as cpool,
            tc.tile_pool(name="xt", bufs=5) as xtpool,
            tc.tile_pool(name="x", bufs=4) as xpool,
            tc.tile_pool(name="ut", bufs=3) as utpool,
            tc.tile_pool(name="sm", bufs=3) as smpool,
            tc.tile_pool(name="p_u", bufs=3, space="PSUM") as p_u_pool,
            tc.tile_pool(name="p_small", bufs=1, space="PSUM") as p_small_pool,
            tc.tile_pool(name="p_o", bufs=2, space="PSUM") as p_o_pool,
        ):
            # ---- warmup tile: shipped as a tiny input, FIRST on the scalar
            # ring (descriptor-gen there is ~0.65us vs ~1.9us on SP; SP's
            # first trigger must be the batch-0 xt load). A memset would
            # chain behind the last engine's ~7us program load; Q7 ext ops
            # would cost a ~6us IRAM library load. ----
            warm_sb = cpool.tile([P, P], BF16)
            nc.scalar.dma_start(warm_sb[:], warm_d[:])
            ones_sb = cpool.tile([P, 1], BF16)
            nc.vector.memset(ones_sb[:], 1.0)

            # consts ride the scalar (ACT) ring, split so the first GEMM can
            # start as soon as its half + xt chunk 0 land
            wb_sb = cpool.tile([P, NK * A], BF16)  # [h_local, (k a)]
            for h2 in range(2):
                nc.scalar.dma_start(
                    wb_sb[:, h2 * 4 * A : (h2 + 1) * 4 * A].rearrange(
                        "p (k a) -> p k a", k=4
                    ),
                    wb_d.rearrange("(k p) a -> p k a", p=P)[
                        :, 4 * h2 : 4 * (h2 + 1)
                    ],
                )
            u_sb = cpool.tile([P, 2 * NA], BF16)  # [a_local, (a_chunk, 0)]
            nc.scalar.dma_start(u_sb[:], u_d[:])

            # output rows for ALL batches, partition 0: col = b*H + h
            o_all = cpool.tile([1, B_LOC * H], F32)

            # PE warm-up until the first wb/xt chunks land
            p_warm = p_u_pool.tile([P, S], F32, tag="p_u")
            for i in range(20):
                nc.tensor.matmul(
                    p_warm[:, 0:P], warm_sb[:], warm_sb[:],
                    start=(i == 0), stop=(i == 19),
                )

            state = {}  # per-batch tiles for the two deferred stages

            def emit_logit(b_):
                ut_sb = state[b_]["ut"]
                # ---- 3. logit^T [s_loc, 2t] (pad cols stay zero) ----
                p_small = p_small_pool.tile([P, 2 * NT + NT], F32, tag="p_sm")
                p_lt = p_small[:, 0 : 2 * NT]
                for t in range(NT):
                    for a in range(NA):
                        nc.tensor.matmul(
                            p_lt[:, 2 * t : 2 * t + 2],
                            ut_sb[:, a * S + t * P : a * S + (t + 1) * P],
                            u_sb[:, 2 * a : 2 * a + 2],
                            start=(a == 0),
                            stop=(a == NA - 1),
                        )
                # ---- 4. wt = exp(logit^T) (unnormalized; 1/sum applied at
                # evacuation) ----
                wt = smpool.tile([P, 2 * NT], BF16, tag="wt")
                nc.scalar.activation(wt[:], p_lt[:], AF.Exp)
                state[b_]["wt"] = wt
                state[b_]["p_small"] = p_small

            def emit_step7(b_):
                wt = state[b_]["wt"]
                x_sb = state[b_]["x"]
                p_small = state[b_]["p_small"]
                # ---- 5. softmax sum: ones.T @ wt(even cols) -> 1/sum ----
                p_s = p_small[0:1, 2 * NT : 2 * NT + NT]
                nc.tensor.matmul(
                    p_s,
                    ones_sb[:],
                    wt[:].rearrange("p (t two) -> p t two", two=2)[:, :, 0],
                    start=True,
                    stop=True,
                )
                ssum = smpool.tile([1, 1], F32, tag="ssum")
                nc.vector.tensor_reduce(ssum[:], p_s, axis=AX.X, op=ALU.add)
                rs = smpool.tile([1, 1], F32, tag="rs")
                nc.vector.reciprocal(rs[:], ssum[:])
                # ---- 6+7. weighted sum on PE, each psum half evacuated
                # (with the 1/sum scale folded in) as soon as it stops, so
                # the ACT evac overlaps PE's second half ----
                p_o = p_o_pool.tile([1, 2 * S], F32, tag="p_o")
                for n in range(2):
                    for t in range(NT):
                        nc.tensor.matmul(
                            p_o[:, n * S : (n + 1) * S],
                            wt[:, 2 * t : 2 * t + 1],
                            x_sb[:, t * H + n * S : t * H + (n + 1) * S],
                            start=(t == 0),
                            stop=(t == NT - 1),
                        )
                    if n == 0:
                        nc.scalar.activation(
                            o_all[:, b_ * H : b_ * H + S], p_o[:, 0:S],
                            AF.Copy, scale=rs[:],
                        )
                    else:
                        nc.vector.tensor_scalar_mul(
                            o_all[:, b_ * H + S : (b_ + 1) * H],
                            p_o[:, S : 2 * S], rs[:],
                        )
                del state[b_]

            for b in range(B_LOC):
                # ---- 1. xt on the SP ring (its queue carries nothing else,
                # so batch 0's halves stream concurrently with the scalar
                # ring's warm/wb transfers), x natural on the scalar ring ----
                xt_all = xtpool.tile([P, NK * S], BF16, tag="xt")
                for k0, kn in ((0, 4), (4, 4)):
                    nc.sync.dma_start(
                        xt_all[:, k0 * S : (k0 + kn) * S].rearrange(
                            "p (k s) -> p k s", k=kn
                        ),
                        xt_d[b, :, k0 : k0 + kn],
                    )
                xt_tiles = [xt_all[:, k * S : (k + 1) * S] for k in range(NK)]
                x_sb = xpool.tile([P, NT * H], BF16, tag="x")
                nc.scalar.dma_start(
                    x_sb[:].rearrange("p (t h) -> p t h", t=NT),
                    x_d[b],
                )

                # ---- 2. GEMM1 + tanh per a-half ----
                ut_sb = utpool.tile([P, 2 * S], BF16, tag="ut")
                for a in range(NA):
                    p_u = p_u_pool.tile([P, S], F32, tag="p_u")
                    for k in range(NK):
                        nc.tensor.matmul(
                            p_u[:],
                            wb_sb[:, k * A + a * P : k * A + (a + 1) * P],
                            xt_tiles[k],
                            start=(k == 0),
                            stop=(k == NK - 1),
                        )
                    nc.scalar.activation(
                        ut_sb[:, a * S : (a + 1) * S], p_u[:], AF.Tanh
                    )
                state[b] = {"ut": ut_sb, "x": x_sb}

                # two-level deferral: logit one batch late, step7 two late
                if b >= 1:
                    emit_logit(b - 1)
                if b >= 2:
                    emit_step7(b - 2)

            emit_logit(B_LOC - 1)
            emit_step7(B_LOC - 2)
            emit_step7(B_LOC - 1)

            # ---- single output DMA for all batches (32KB contiguous); the
            # scalar ring's queue has drained by then and its descriptor
            # generation is faster than SP's ----
            nc.scalar.dma_start(
                out_d.rearrange("b h -> (b h)").unsqueeze(0), o_all[:]
            )

    nc.compile()
    return nc


def get_nc():
    if "nc" not in _CACHE:
        _CACHE["nc"] = _build()
    return _CACHE["nc"]


def make_in_maps(inputs, attention_w, attention_u, attention_b):
    import ml_dtypes

    bf16 = ml_dtypes.bfloat16
    x0 = np.asarray(inputs, dtype=np.float32).astype(bf16)
    # natural copy, partition-major: x[b, s_local, t, h] (8KB lines)
    x = np.ascontiguousarray(
        x0.reshape(B_FULL, NT, P, H).transpose(0, 2, 1, 3)
    )
    # transposed copy, partition-major: xt[b, h_local, k, s] (8KB lines)
    xt = np.ascontiguousarray(
        x0.reshape(B_FULL, S, NK, P).transpose(0, 3, 2, 1)
    )
    w = np.asarray(attention_w, dtype=np.float32)
    u = np.asarray(attention_u, dtype=np.float32)
    b = np.asarray(attention_b, dtype=np.float32)
    wb = np.ascontiguousarray(w + b[None, :]).astype(bf16)
    u4 = np.zeros((P, 2 * NA), dtype=np.float32)  # [a_local, (a_chunk, 0)]
    for a in range(NA):
        u4[:, 2 * a] = u[a * P : (a + 1) * P, 0]
    u4 = u4.astype(bf16)
    warm = np.ones((P, P), dtype=bf16)
    in_maps = []
    for c in range(N_CORES):
        in_maps.append(
            {
                "x": x[c * B_LOC : (c + 1) * B_LOC],
                "xt": xt[c * B_LOC : (c + 1) * B_LOC],
                "wb": wb,
                "u4": u4,
                "warm": warm,
            }
        )
    return in_maps


def kernel(inputs, attention_w, attention_u, attention_b):
    from concourse.bass_utils import run_bass_kernel_spmd

    nc = get_nc()
    in_maps = make_in_maps(inputs, attention_w, attention_u, attention_b)
    res = run_bass_kernel_spmd(nc, in_maps, list(range(N_CORES)))
    out = np.concatenate(
        [res.results[c]["out"] for c in range(N_CORES)], axis=0
    ).astype(np.float32)
    return out


# revision 70
# speedup vs baseline: 1.0204x; 1.0204x over previous
"""Trainium2 Bass kernel for nn_ATTLayer (attention pooling).

Reference computation (full input [64, 512, 1024] fp32):
    wb    = attention_w + attention_b          # [1024, 256] (b broadcast over rows)
    u_t   = tanh(inputs @ wb)                  # [64, 512, 256]
    logit = u_t @ attention_u                  # [64, 512]
    w     = softmax(logit, axis=1)             # softmax over seq
    out   = sum_s w[:, s] * inputs[:, s, :]    # [64, 1024]

Sharding: data-parallel over batch — 8 batches per core on 8 NeuronCores, no
collectives. Tiny params (wb = W + b, u) are precomputed/replicated on host.

All heavy math runs on PE (HW-measured: back-to-back matmuls stream at full
rate; DVE reduces cost ~0.7us per [128,512] on HW, so the weighted sum stays
on PE). x is uploaded in TWO layouts (block-transposed xt for GEMM1, natural
x for the weighted sum) on separate HWDGE rings (~290 / ~220 GB/s each,
HW-measured, transfers run on DMA engines, not the issuing queue).

Per local batch b (two-level software pipelining keeps PE stall-free:
logit^T(b) is emitted after GEMM1(b+1), softmax-sum+step7(b) after
GEMM1(b+2), so the ACT-exp chain for b overlaps a full GEMM):
  1. DMA xt (SP ring), x natural (scalar ring)
  2. GEMM1: p_u[a] += wb[k,a].T @ xt[k], 8 k-chunks; tanh per a-half (ACT)
  3. logit^T [s_loc, 2t] via 8 small matmuls (ut chunk stationary, u padded
     with zero columns for the even-free-dim rule)
  4. wt = exp(logit^T) on ACT, unnormalized. No max-subtraction:
     |logit| <= ~20 so exp stays finite in fp32/bf16.
  5. softmax sum = ones.T @ wt (PE) -> DVE reduce -> reciprocal
  6. step7: p_o[1, h] += wt[:, t].T @ x[t-chunk, h]  (8 matmuls)
  7. evacuation applies the 1/sum scale (ACT half / DVE half); ONE output
     DMA for all batches at the end on the then-idle SP ring.

Startup notes (HW-measured): ~8.5us of engine program loads precede any
work; the warm tile is shipped as a tiny SP-ring input so PE warm-up only
waits on SP (a memset would chain behind the last engine's program load,
and any Q7 ext-ISA op would add a ~6us IRAM library load to startup).

bf16 matmul operands / fp32 accumulation end-to-end rel err ~7e-3.
"""

import numpy as np

N_CORES = 8
B_FULL = 64
B_LOC = B_FULL // N_CORES  # 8 batches per core
S = 512
H = 1024
A = 256
P = 128
NT = S // P      # 4 s-tiles per batch
NK = H // P      # 8 h-chunks
NA = A // P      # 2 a-chunks

_CACHE = {}


def _build():
    import concourse.bacc as bacc
    import concourse.mybir as mybir
    import concourse.tile as tile

    F32 = mybir.dt.float32
    BF16 = mybir.dt.bfloat16
    AF = mybir.ActivationFunctionType
    AX = mybir.AxisListType
    ALU = mybir.AluOpType

    nc = bacc.Bacc("TRN2", target_bir_lowering=False, debug=False)

    # xt is partition-major on the host: each partition's 8 k-chunks are one
    # 8KB contiguous run, so DMA descriptors cover 2-8KB lines (~1.5x rate)
    xt_d = nc.dram_tensor("xt", [B_LOC, P, NK, S], BF16, kind="ExternalInput").ap()
    x_d = nc.dram_tensor("x", [B_LOC, P, NT, H], BF16, kind="ExternalInput").ap()
    wb_d = nc.dram_tensor("wb", [H, A], BF16, kind="ExternalInput").ap()
    u_d = nc.dram_tensor("u4", [P, 2 * NA], BF16, kind="ExternalInput").ap()
    warm_d = nc.dram_tensor("warm", [P, P], BF16, kind="ExternalInput").ap()
    out_d = nc.dram_tensor("out", [B_LOC, H], F32, kind="ExternalOutput").ap()

    with tile.TileContext(nc) as tc:
        with (
            tc.tile_pool(name="const", bufs=1) as cpool,
            tc.tile_pool(name="xt", bufs=5) as xtpool,
            tc.tile_pool(name="x", bufs=4) as xpool,
            tc.tile_pool(name="ut", bufs=3) as utpool,
            tc.tile_pool(name="sm", bufs=3) as smpool,
            tc.tile_pool(name="p_u", bufs=3, space="PSUM") as p_u_pool,
            tc.tile_pool(name="p_small", bufs=1, space="PSUM") as p_small_pool,
            tc.tile_pool(name="p_o", bufs=2, space="PSUM") as p_o_pool,
        ):
            # ---- warmup tile: shipped as a tiny input, FIRST on the scalar
            # ring (descriptor-gen there is ~0.65us vs ~1.9us on SP; SP's
            # first trigger must be the batch-0 xt load). A memset would
            # chain behind the last engine's ~7us program load; Q7 ext ops
            # would cost a ~6us IRAM library load. ----
            warm_sb = cpool.tile([P, P], BF16)
            nc.scalar.dma_start(warm_sb[:], warm_d[:])
            ones_sb = cpool.tile([P, 1], BF16)
            nc.vector.memset(ones_sb[:], 1.0)

            # consts ride the scalar (ACT) ring, split so the first GEMM can
            # start as soon as its half + xt chunk 0 land
            wb_sb = cpool.tile([P, NK * A], BF16)  # [h_local, (k a)]
            for h2 in range(2):
                nc.scalar.dma_start(
                    wb_sb[:, h2 * 4 * A : (h2 + 1) * 4 * A].rearrange(
                        "p (k a) -> p k a", k=4
                    ),
                    wb_d.rearrange("(k p) a -> p k a", p=P)[
                        :, 4 * h2 : 4 * (h2 + 1)
                    ],
                )
            u_sb = cpool.tile([P, 2 * NA], BF16)  # [a_local, (a_chunk, 0)]
            nc.scalar.dma_start(u_sb[:], u_d[:])

            # output rows for ALL batches, partition 0: col = b*H + h
            o_all = cpool.tile([1, B_LOC * H], F32)

            # PE warm-up until the first wb/xt chunks land
            p_warm = p_u_pool.tile([P, S], F32, tag="p_u")
            for i in range(20):
                nc.tensor.matmul(
                    p_warm[:, 0:P], warm_sb[:], warm_sb[:],
                    start=(i == 0), stop=(i == 19),
                )

            state = {}  # per-batch tiles for the two deferred stages

            def emit_logit(b_):
                ut_sb = state[b_]["ut"]
                # ---- 3. logit^T [s_loc, 2t] (pad cols stay zero) ----
                p_small = p_small_pool.tile([P, 2 * NT + NT], F32, tag="p_sm")
                p_lt = p_small[:, 0 : 2 * NT]
                for t in range(NT):
                    for a in range(NA):
                        nc.tensor.matmul(
                            p_lt[:, 2 * t : 2 * t + 2],
                            ut_sb[:, a * S + t * P : a * S + (t + 1) * P],
                            u_sb[:, 2 * a : 2 * a + 2],
                            start=(a == 0),
                            stop=(a == NA - 1),
                        )
                # ---- 4. wt = exp(logit^T) (unnormalized; 1/sum applied at
                # evacuation) ----
                wt = smpool.tile([P, 2 * NT], BF16, tag="wt")
                nc.scalar.activation(wt[:], p_lt[:], AF.Exp)
                state[b_]["wt"] = wt
                state[b_]["p_small"] = p_small

            def emit_step7(b_):
                wt = state[b_]["wt"]
                x_sb = state[b_]["x"]
                p_small = state[b_]["p_small"]
                # ---- 5. softmax sum: ones.T @ wt(even cols) -> 1/sum ----
                p_s = p_small[0:1, 2 * NT : 2 * NT + NT]
                nc.tensor.matmul(
                    p_s,
                    ones_sb[:],
                    wt[:].rearrange("p (t two) -> p t two", two=2)[:, :, 0],
                    start=True,
                    stop=True,
                )
                ssum = smpool.tile([1, 1], F32, tag="ssum")
                nc.vector.tensor_reduce(ssum[:], p_s, axis=AX.X, op=ALU.add)
                rs = smpool.tile([1, 1], F32, tag="rs")
                nc.vector.reciprocal(rs[:], ssum[:])
                # ---- 6. weighted sum on PE: p_o[1, h] over 4 t-chunks ----
                p_o = p_o_pool.tile([1, 2 * S], F32, tag="p_o")
                for n in range(2):
                    for t in range(NT):
                        nc.tensor.matmul(
                            p_o[:, n * S : (n + 1) * S],
                            wt[:, 2 * t : 2 * t + 1],
                            x_sb[:, t * H + n * S : t * H + (n + 1) * S],
                            start=(t == 0),
                            stop=(t == NT - 1),
                        )
                # ---- 7. evacuate with the 1/sum scale folded in (slice
                # deps let the ACT half start as soon as psum half 0 stops) ----
                nc.scalar.activation(
                    o_all[:, b_ * H : b_ * H + S], p_o[:, 0:S],
                    AF.Copy, scale=rs[:],
                )
                nc.vector.tensor_scalar_mul(
                    o_all[:, b_ * H + S : (b_ + 1) * H], p_o[:, S : 2 * S],
                    rs[:],
                )
                del state[b_]

            for b in range(B_LOC):
                # ---- 1. xt on the SP ring (its queue carries nothing else,
                # so batch 0's halves stream concurrently with the scalar
                # ring's warm/wb transfers), x natural on the scalar ring ----
                xt_all = xtpool.tile([P, NK * S], BF16, tag="xt")
                for k0, kn in ((0, 4), (4, 4)):
                    nc.sync.dma_start(
                        xt_all[:, k0 * S : (k0 + kn) * S].rearrange(
                            "p (k s) -> p k s", k=kn
                        ),
                        xt_d[b, :, k0 : k0 + kn],
                    )
                xt_tiles = [xt_all[:, k * S : (k + 1) * S] for k in range(NK)]
                x_sb = xpool.tile([P, NT * H], BF16, tag="x")
                nc.scalar.dma_start(
                    x_sb[:].rearrange("p (t h) -> p t h", t=NT),
                    x_d[b],
                )

                # ---- 2. GEMM1 + tanh per a-half ----
                ut_sb = utpool.tile([P, 2 * S], BF16, tag="ut")
                for a in range(NA):
                    p_u = p_u_pool.tile([P, S], F32, tag="p_u")
                    for k in range(NK):
                        nc.tensor.matmul(
                            p_u[:],
                            wb_sb[:, k * A + a * P : k * A + (a + 1) * P],
                            xt_tiles[k],
                            start=(k == 0),
                            stop=(k == NK - 1),
                        )
                    nc.scalar.activation(
                        ut_sb[:, a * S : (a + 1) * S], p_u[:], AF.Tanh
                    )
                state[b] = {"ut": ut_sb, "x": x_sb}

                # two-level deferral: logit one batch late, step7 two late
                if b >= 1:
                    emit_logit(b - 1)
                if b >= 2:
                    emit_step7(b - 2)

            emit_logit(B_LOC - 1)
            emit_step7(B_LOC - 2)
            emit_step7(B_LOC - 1)

            # ---- single output DMA for all batches (32KB contiguous); the
            # scalar ring's queue has drained by then and its descriptor
            # generation is faster than SP's ----
            nc.scalar.dma_start(
                out_d.rearrange("b h -> (b h)").unsqueeze(0), o_all[:]
            )

    nc.compile()
    return nc


def get_nc():
    if "nc" not in _CACHE:
        _CACHE["nc"] = _build()
    return _CACHE["nc"]


def make_in_maps(inputs, attention_w, attention_u, attention_b):
    import ml_dtypes

    bf16 = ml_dtypes.bfloat16
    x0 = np.asarray(inputs, dtype=np.float32).astype(bf16)
    # natural copy, partition-major: x[b, s_local, t, h] (8KB lines)
    x = np.ascontiguousarray(
        x0.reshape(B_FULL, NT, P, H).transpose(0, 2, 1, 3)
    )
    # transposed copy, partition-major: xt[b, h_local, k, s] (8KB lines)
    xt = np.ascontiguousarray(
        x0.reshape(B_FULL, S, NK, P).transpose(0, 3, 2, 1)
    )
    w = np.asarray(attention_w, dtype=np.float32)
    u = np.asarray(attention_u, dtype=np.float32)
    b = np.asarray(attention_b, dtype=np.float32)
    wb = np.ascontiguousarray(w + b[None, :]).astype(bf16)
    u4 = np.zeros((P, 2 * NA), dtype=np.float32)  # [a_local, (a_chunk, 0)]
    for a in range(NA):
        u4[:, 2 * a] = u[a * P : (a + 1) * P, 0]
    u4 = u4.astype(bf16)
    warm = np.ones((P, P), dtype=bf16)
    in_maps = []
    for c in range(N_CORES):
        in_maps.append(
            {
                "x": x[c * B_LOC : (c + 1) * B_LOC],
                "xt": xt[c * B_LOC : (c + 1) * B_LOC],
                "wb": wb,
                "u4": u4,
                "warm": warm,
            }
        )
    return in_maps


def kernel(inputs, attention_w, attention_u, attention_b):
    from concourse.bass_utils import run_bass_kernel_spmd

    nc = get_nc()
    in_maps = make_in_maps(inputs, attention_w, attention_u, attention_b)
    res = run_bass_kernel_spmd(nc, in_maps, list(range(N_CORES)))
    out = np.concatenate(
        [res.results[c]["out"] for c in range(N_CORES)], axis=0
    ).astype(np.float32)
    return out
